# revision 1
# baseline (speedup 1.0000x reference)
"""DeepseekV32 MLA-style attention on 8 Trainium2 NeuronCores (Bass/Tile), v2.

Sharding identical to v1 (tensor-parallel over heads for attention, row-shard
for the A projections / q_b / o_proj), restructured for the TRN2 timeline:

- hidden arrives host-pre-transposed (hsT [H, SPC]) — no on-chip hs transposes.
- PE warm-up chains before cold matmul sections (clock p-state ramp).
- ckv -> AllGather issued as early as possible; weights stream in behind it.
- q is roped, scaled by 16 (folded into wq_b, undone in the exp), cast to
  fp8e4m3, transposed on-chip, and AllToAll'd in fp8 (halves that collective).
- Attention computes scores TRANSPOSED (scoresT[k, q] = K Q^T) so PV consumes
  row-major V directly; no per-tile probs transposes/copies. The scores matmul
  runs fp8 DoubleRow with nope+rope stacked into the 2-plane contraction
  (one matmul per chunk at 0.5 cycles/row).
- Softmax is unnormalized: a ones-vector matmul per chunk accumulates the
  denominators in PSUM [1, q]; partition_broadcast + reciprocal + one multiply
  normalize attnT before the output AllToAll. Causal mask = in-place
  affine_select(fill=0) on the exp output's diagonal block (Pool engine).
- Output AllToAll split into even/odd q-tile halves; o_proj per half overlaps
  the other half's collective.
"""
import sys

sys.path.insert(0, "/opt/trn_rl_repo")

import numpy as np
import ml_dtypes
from contextlib import ExitStack

import concourse.bass as bass
import concourse.tile as tile
import concourse.mybir as mybir
from concourse import bacc
from concourse.masks import make_identity
from concourse.bass_utils import run_bass_kernel_spmd

BF16 = mybir.dt.bfloat16
F32 = mybir.dt.float32
FP8 = mybir.dt.float8e4
AF = mybir.ActivationFunctionType
DRMODE = mybir.MatmulPerfMode.DoubleRow

NC = 8            # cores
B, S, H = 1, 2048, 2048
NH = 16           # heads
QLR = 1536        # q lora rank
KVLR = 512        # kv lora rank
DR = 64           # rope dim
DN = 128          # nope dim
DV = 128          # v dim
DQK = DN + DR     # 192
EPS = 1e-6
HPC = NH // NC    # heads per core = 2
SPC = S // NC     # seq rows per core = 256
NT = S // 128     # 16 q/k tiles of the full sequence
Q_SCALE = 16.0    # folded into wq_b for fp8 headroom; undone in the exp

_CACHED = {}


def _ts(i, n):
    return slice(i * n, (i + 1) * n)


def build(use_collectives=True):
    nc = bacc.Bacc("TRN2", target_bir_lowering=False, debug=False,
                   num_devices=NC)

    def collective(kind, in_ap, out_ap):
        if use_collectives:
            nc.gpsimd.collective_compute(
                kind, mybir.AluOpType.bypass, replica_groups=[list(range(NC))],
                ins=[in_ap.opt()], outs=[out_ap.opt()])
        else:
            n = in_ap.shape[0]
            nc.sync.dma_start(out=out_ap[0:n], in_=in_ap)

    # ---- kernel I/O ----
    hsT_d = nc.dram_tensor("hsT", [H, SPC], BF16, kind="ExternalInput").ap()
    cos_d = nc.dram_tensor("cosr", [SPC, DR], F32, kind="ExternalInput").ap()
    sin_d = nc.dram_tensor("sinr", [SPC, DR], F32, kind="ExternalInput").ap()
    wqa_d = nc.dram_tensor("wqa", [H, QLR], BF16, kind="ExternalInput").ap()
    wkva_d = nc.dram_tensor("wkva", [H, KVLR + DR], BF16,
                            kind="ExternalInput").ap()
    wqb_d = nc.dram_tensor("wqb", [QLR, NH * DQK], BF16,
                           kind="ExternalInput").ap()
    # host-reordered: [KVLR, k_h0 | k_h1 | v_h0 | v_h1]
    wkvb_d = nc.dram_tensor("wkvb", [KVLR, HPC * (DN + DV)], BF16,
                            kind="ExternalInput").ap()
    wo_d = nc.dram_tensor("wo", [NH * DV, H], BF16, kind="ExternalInput").ap()
    out_d = nc.dram_tensor("out", [SPC, H], F32, kind="ExternalOutput").ap()

    # ---- collective buffers ----
    ag_in = nc.dram_tensor("ag_in", [KVLR + DR, SPC], BF16).ap()
    ag_out = nc.dram_tensor("ag_out", [NC * (KVLR + DR), SPC], BF16,
                            addr_space="Shared").ap()
    a2aq_in = nc.dram_tensor("a2aq_in", [NH * DQK, SPC], FP8).ap()
    a2aq_out = nc.dram_tensor("a2aq_out", [NH * DQK, SPC], FP8).ap()
    a2ao_in = [nc.dram_tensor(f"a2ao{x}_in", [NH * DV, SPC // 2], BF16).ap()
               for x in "EO"]
    a2ao_out = [nc.dram_tensor(f"a2ao{x}_out", [NH * DV, SPC // 2], BF16).ap()
                for x in "EO"]

    with tile.TileContext(nc) as tc, ExitStack() as ctx:
        singles = ctx.enter_context(tc.tile_pool(name="singles", bufs=1))
        small = ctx.enter_context(tc.tile_pool(name="small", bufs=4))

        ident = singles.tile([128, 128], BF16)
        make_identity(nc, ident)
        eps_t = singles.tile([128, 1], F32)
        nc.vector.memset(eps_t, float(EPS))
        ones_t = singles.tile([128, 1], BF16)
        nc.vector.memset(ones_t, 1.0)
        ident8 = singles.tile([128, 128], FP8)
        nc.vector.tensor_copy(out=ident8, in_=ident)

        def warm(n, tag):
            # dummy transpose chain to ramp / keep the PE clock
            with tc.tile_pool(name=f"warm{tag}", bufs=2, space="PSUM") as wp:
                for _ in range(n):
                    w_p = wp.tile([128, 128], BF16, tag="w", name="w_p")
                    nc.tensor.transpose(out=w_p, in_=ident, identity=ident)

        # rope cos/sin for this core's rows
        cos_sb, sin_sb = [], []
        for st in range(2):
            c_t = small.tile([128, DR], F32, tag="cos", bufs=2)
            s_t = small.tile([128, DR], F32, tag="sin", bufs=2)
            nc.sync.dma_start(out=c_t, in_=cos_d[_ts(st, 128), :])
            nc.sync.dma_start(out=s_t, in_=sin_d[_ts(st, 128), :])
            cos_sb.append(c_t)
            sin_sb.append(s_t)

        def rope_pair(out_ap, xe, xo, c0, s0, c1, s1, shape, tagp="r"):
            """out0 = xe*c0 - xo*s0 ; out1 = xo*c1 + xe*s1."""
            t0 = small.tile(shape, F32, tag=tagp + "0", bufs=2, name="t0")
            t1 = small.tile(shape, F32, tag=tagp + "1", bufs=2, name="t1")
            o0, o1 = out_ap
            nc.vector.tensor_mul(t0, xe, c0)
            nc.vector.tensor_mul(t1, xo, s0)
            nc.vector.tensor_sub(o0, t0, t1)
            nc.vector.tensor_mul(t0, xo, c1)
            nc.vector.tensor_mul(t1, xe, s1)
            nc.vector.tensor_add(o1, t0, t1)

        warm(46, "a")  # ramp + cover the input DMA latency

        # ================= phase 1: ckv -> AllGather =================
        ph12 = ExitStack()  # hsT lives through phase 2
        hsTp = ph12.enter_context(tc.tile_pool(name="hsTp", bufs=1))
        DKV = KVLR + DR
        nh = H // 128
        hsT = hsTp.tile([128, nh * SPC], BF16, tag="hsT", name="hsT")
        src = bass.AP(tensor=hsT_d.tensor, offset=0,
                      ap=[[SPC, 128], [128 * SPC, nh], [1, SPC]])
        nc.sync.dma_start(out=hsT.rearrange("p (t s) -> p t s", t=nh),
                          in_=src)

        with tc.tile_pool(name="ph1", bufs=1) as ph1, \
             tc.tile_pool(name="ps1", bufs=2, space="PSUM") as ps1:
            wkva = ph1.tile([128, nh * DKV], BF16, tag="wkva", name="wkva")
            for hf in range(2):
                srcw = bass.AP(
                    tensor=wkva_d.tensor, offset=hf * 8 * 128 * DKV,
                    ap=[[DKV, 128], [128 * DKV, 8], [1, DKV]])
                nc.sync.dma_start(
                    out=wkva[:, hf * 8 * DKV:(hf + 1) * 8 * DKV].rearrange(
                        "p (t d) -> p t d", t=8),
                    in_=srcw)

            ckvn_bf, krot_bf = [], []
            for st in range(2):
                ckv_p = ps1.tile([128, KVLR], F32, tag="ckv", bufs=2,
                                 name="ckv_p")
                ckr_p = ps1.tile([128, DR], F32, tag="ckr", bufs=2,
                                 name="ckr_p")
                for ht in range(nh):
                    nc.tensor.matmul(
                        out=ckv_p,
                        lhsT=hsT[:, ht * SPC + st * 128:ht * SPC + st * 128 + 128],
                        rhs=wkva[:, ht * DKV:ht * DKV + KVLR],
                        start=(ht == 0), stop=(ht == nh - 1))
                for ht in range(nh):
                    nc.tensor.matmul(
                        out=ckr_p,
                        lhsT=hsT[:, ht * SPC + st * 128:ht * SPC + st * 128 + 128],
                        rhs=wkva[:, ht * DKV + KVLR:ht * DKV + DKV],
                        start=(ht == 0), stop=(ht == nh - 1))
                # rmsnorm over KVLR (Act reads PSUM directly)
                sq = small.tile([128, KVLR], BF16, tag="sqscr", bufs=2,
                                name="sq")
                ssq = small.tile([128, 1], F32, tag="ssq", name="ssq")
                nc.scalar.activation(out=sq, in_=ckv_p, func=AF.Square,
                                     accum_out=ssq)
                rstd = small.tile([128, 1], F32, tag="rstd", name="rstd")
                nc.scalar.activation(out=rstd, in_=ssq, func=AF.Sqrt,
                                     scale=1.0 / KVLR, bias=eps_t)
                nc.vector.reciprocal(out=rstd, in_=rstd)
                cn_t = ph1.tile([128, KVLR], BF16, tag=f"ckvn{st}",
                                name="cn_t")
                nc.vector.tensor_scalar_mul(cn_t, ckv_p, rstd)
                ckvn_bf.append(cn_t)
                kr_t = ph1.tile([128, DR], BF16, tag=f"krot{st}", name="kr_t")
                rope_pair(
                    (kr_t[:, 0:DR // 2], kr_t[:, DR // 2:DR]),
                    ckr_p[:, 0:DR:2], ckr_p[:, 1:DR:2],
                    cos_sb[st][:, 0:DR // 2], sin_sb[st][:, 0:DR // 2],
                    cos_sb[st][:, DR // 2:DR], sin_sb[st][:, DR // 2:DR],
                    [128, DR // 2], tagp="rk")
                krot_bf.append(kr_t)

            # transposes -> X^T stage tiles, then DMA to ag_in
            xT = ph1.tile([128, 4 * SPC], BF16, tag="xT", name="xT")
            xTr = ph1.tile([64, SPC], BF16, tag="xTr", name="xTr")
            for kt in range(KVLR // 128):
                p_t = ps1.tile([128, SPC], BF16, tag="tp", name="p_t")
                for st in range(2):
                    nc.tensor.transpose(out=p_t[:, _ts(st, 128)],
                                        in_=ckvn_bf[st][:, _ts(kt, 128)],
                                        identity=ident)
                nc.scalar.copy(out=xT[:, _ts(kt, SPC)], in_=p_t)
            pr_t = ps1.tile([64, SPC], BF16, tag="tpr", bufs=1, name="pr_t")
            for st in range(2):
                nc.tensor.transpose(out=pr_t[:, _ts(st, 128)],
                                    in_=krot_bf[st], identity=ident)
            nc.scalar.copy(out=xTr, in_=pr_t)
            dst = bass.AP(tensor=ag_in.tensor, offset=0,
                          ap=[[SPC, 128], [128 * SPC, 4], [1, SPC]])
            nc.scalar.dma_start(
                out=dst, in_=xT.rearrange("p (k s) -> p k s", k=4))
            nc.scalar.dma_start(out=ag_in[KVLR:KVLR + DR, :], in_=xTr)

            collective("AllGather", ag_in, ag_out)

        # ========== phase 2: cq, q_b, rope, qT, AllToAll(q) fp8 ==========
        with tc.tile_pool(name="ph2", bufs=1) as ph2, \
             tc.tile_pool(name="wqap", bufs=4) as wqap:
            # --- cq = hsT^T @ wqa, both row-tiles, wqa streamed ---
            cqn_bf = []
            with tc.tile_pool(name="ps2a", bufs=1, space="PSUM") as ps2a:
                cq_ps = [ps2a.tile([128, QLR], F32, tag=f"cq{st}",
                                   name="cq_p") for st in range(2)]
                for ht in range(nh):
                    wq_t = wqap.tile([128, QLR], BF16, tag="wqa",
                                     name="wq_t")
                    nc.sync.dma_start(out=wq_t, in_=wqa_d[_ts(ht, 128), :])
                    for st in range(2):
                        for rb in range(QLR // 512):
                            nc.tensor.matmul(
                                out=cq_ps[st][:, _ts(rb, 512)],
                                lhsT=hsT[:, ht * SPC + st * 128:
                                         ht * SPC + st * 128 + 128],
                                rhs=wq_t[:, _ts(rb, 512)],
                                start=(ht == 0), stop=(ht == nh - 1))
                for st in range(2):
                    sqq = ph2.tile([128, QLR], BF16, tag="sqq", bufs=2,
                                   name="sqq")
                    ssq = small.tile([128, 1], F32, tag="ssq", name="ssq")
                    nc.scalar.activation(out=sqq, in_=cq_ps[st],
                                         func=AF.Square, accum_out=ssq)
                    rstd = small.tile([128, 1], F32, tag="rstd", name="rstd")
                    nc.scalar.activation(out=rstd, in_=ssq, func=AF.Sqrt,
                                         scale=1.0 / QLR, bias=eps_t)
                    nc.vector.reciprocal(out=rstd, in_=rstd)
                    cn_t = ph2.tile([128, QLR], BF16, tag=f"cqn{st}",
                                    name="cn_t")
                    nc.vector.tensor_scalar_mul(cn_t, cq_ps[st], rstd)
                    cqn_bf.append(cn_t)

                # cqnT: 12 tiles [128, 256] (st0|st1)
                cqnT = []
                for rt in range(QLR // 128):
                    p_t = ps2a.tile([128, SPC], BF16, tag="tp2", bufs=2,
                                    name="p_t")
                    for st in range(2):
                        nc.tensor.transpose(out=p_t[:, _ts(st, 128)],
                                            in_=cqn_bf[st][:, _ts(rt, 128)],
                                            identity=ident)
                    cT_t = ph2.tile([128, SPC], BF16, tag=f"cqnT{rt}",
                                    name="cT_t")
                    nc.scalar.copy(out=cT_t, in_=p_t)
                    cqnT.append(cT_t)

            # wqb resident (loaded during cq)
            wqb = ph2.tile([128, (QLR // 128) * NH * DQK], BF16, tag="wqb",
                           name="wqb")
            NQB = NH * DQK
            for i in range(3):
                srcb = bass.AP(tensor=wqb_d.tensor, offset=i * 4 * 128 * NQB,
                               ap=[[NQB, 128], [128 * NQB, 4], [1, NQB]])
                nc.sync.dma_start(
                    out=wqb[:, i * 4 * NQB:(i + 1) * 4 * NQB].rearrange(
                        "p (t d) -> p t d", t=4),
                    in_=srcb)

            # cos/sin broadcast over heads for the q rope (DVE, stride-0)
            cosq, sinq = [], []
            for st in range(2):
                cq_t = ph2.tile([128, NH, DR], F32, tag=f"cosq{st}",
                                name="cq_t")
                sq_t = ph2.tile([128, NH, DR], F32, tag=f"sinq{st}",
                                name="sq_t")
                csrc = bass.AP(tensor=cos_sb[st].tensor,
                               offset=cos_sb[st].offset,
                               ap=[cos_sb[st].ap[0], [0, NH], [1, DR]])
                ssrc = bass.AP(tensor=sin_sb[st].tensor,
                               offset=sin_sb[st].offset,
                               ap=[sin_sb[st].ap[0], [0, NH], [1, DR]])
                nc.vector.tensor_copy(out=cq_t, in_=csrc)
                nc.vector.tensor_copy(out=sq_t, in_=ssrc)
                cosq.append(cq_t)
                sinq.append(sq_t)

            # --- q_b (wqb resident, st-outer), rope, fp8, transpose ---
            nT_all = ph2.tile([128, NH * SPC], FP8, tag="nT", name="nT_all")
            rT_all = ph2.tile([64, NH * SPC], FP8, tag="rT", name="rT_all")
            nrt = QLR // 128
            with tc.tile_pool(name="ps2b", bufs=1, space="PSUM") as ps2b:
                for st in range(2):
                    q_ps = ps2b.tile([128, NH * DQK], F32, tag="qb",
                                     name="q_p")  # 6 banks
                    for rt in range(nrt):
                        for nb in range(NH * DQK // 512):
                            nc.tensor.matmul(
                                out=q_ps[:, _ts(nb, 512)],
                                lhsT=cqnT[rt][:, _ts(st, 128)],
                                rhs=wqb[:, rt * NQB + nb * 512:
                                        rt * NQB + (nb + 1) * 512],
                                start=(rt == 0), stop=(rt == nrt - 1))
                    qv = q_ps.rearrange("p (h d) -> p h d", h=NH)
                    q_bf = ph2.tile([128, NH, DQK], BF16, tag="qbf",
                                    bufs=2, name="q_bf")
                    nc.vector.tensor_copy(out=q_bf[:, :, 0:DN],
                                          in_=qv[:, :, 0:DN])
                    hw = DR // 2
                    rope_pair(
                        (q_bf[:, :, DN:DN + hw], q_bf[:, :, DN + hw:DQK]),
                        qv[:, :, DN + 0:DQK:2], qv[:, :, DN + 1:DQK:2],
                        cosq[st][:, :, 0:hw], sinq[st][:, :, 0:hw],
                        cosq[st][:, :, hw:DR], sinq[st][:, :, hw:DR],
                        [128, NH, hw], tagp="rq")
                    # qT transposes (fp8): nope [128] + rope [64] per head
                    for h in range(NH):
                        tq = ps2b.tile([128, 256], BF16, tag="tq", bufs=2,
                                       name="tq")
                        nc.tensor.transpose(out=tq[:, 0:128],
                                            in_=q_bf[:, h, 0:DN],
                                            identity=ident)
                        nc.tensor.transpose(out=tq[0:64, 128:256],
                                            in_=q_bf[:, h, DN:DQK],
                                            identity=ident)
                        nc.scalar.copy(
                            out=nT_all[:, h * SPC + st * 128:
                                       h * SPC + st * 128 + 128],
                            in_=tq[:, 0:128])
                        nc.vector.tensor_copy(
                            out=rT_all[:, h * SPC + st * 128:
                                       h * SPC + st * 128 + 128],
                            in_=tq[0:64, 128:256])
            # stage qT -> a2aq_in (2 packed DMAs)
            dstn = bass.AP(tensor=a2aq_in.tensor, offset=0,
                           ap=[[SPC, 128], [DQK * SPC, NH], [1, SPC]])
            nc.scalar.dma_start(
                out=dstn, in_=nT_all.rearrange("p (h s) -> p h s", h=NH))
            dstr = bass.AP(tensor=a2aq_in.tensor, offset=DN * SPC,
                           ap=[[SPC, 64], [DQK * SPC, NH], [1, SPC]])
            nc.scalar.dma_start(
                out=dstr, in_=rT_all.rearrange("p (h s) -> p h s", h=NH))
            collective("AllToAll", a2aq_in, a2aq_out)
        ph12.close()

        # ============ phase 3: k/v build (overlaps A2A-q) ============
        ph3 = ctx.enter_context(tc.tile_pool(name="ph3", bufs=1))
        NKV = HPC * (DN + DV)
        with tc.tile_pool(name="ps3", bufs=1, space="PSUM") as ps3:
            wkvb = ph3.tile([128, (KVLR // 128) * NKV], BF16, tag="wkvb",
                            name="wkvb")
            srck = bass.AP(tensor=wkvb_d.tensor, offset=0,
                           ap=[[NKV, 128], [128 * NKV, KVLR // 128],
                               [1, NKV]])
            nc.sync.dma_start(
                out=wkvb.rearrange("p (t d) -> p t d", t=KVLR // 128),
                in_=srck)

            # gathered X^T: 4 nope k-tile rows + krot, packed reads
            xk = []
            for kt in range(KVLR // 128):
                xk_t = ph3.tile([128, S], BF16, tag=f"xk{kt}", name="xk_t")
                srcx = bass.AP(tensor=ag_out.tensor, offset=kt * 128 * SPC,
                               ap=[[SPC, 128], [(KVLR + DR) * SPC, NC],
                                   [1, SPC]])
                nc.sync.dma_start(
                    out=xk_t.rearrange("p (g s) -> p g s", g=NC), in_=srcx)
                xk.append(xk_t)
            krTr = ph3.tile([64, S], BF16, tag="krTr", name="krTr")
            srcr = bass.AP(tensor=ag_out.tensor, offset=KVLR * SPC,
                           ap=[[SPC, 64], [(KVLR + DR) * SPC, NC], [1, SPC]])
            nc.sync.dma_start(
                out=krTr.rearrange("p (g s) -> p g s", g=NC), in_=srcr)

            # wo loads (big; overlap the A2A-q window)
            wo_sb = []
            for i in range(2):
                wo_t = ph3.tile([128, 8 * H], BF16, tag=f"wo{i}", name="wo_t")
                srco = bass.AP(tensor=wo_d.tensor, offset=i * 1024 * H,
                               ap=[[H, 128], [128 * H, 8], [1, H]])
                nc.sync.dma_start(
                    out=wo_t.rearrange("p (g d) -> p g d", g=8), in_=srco)
                wo_sb.append(wo_t)

            # kT_stack per head: [128, 2*S] fp8, plane0 = nope kT,
            # plane1 = roped krT (zeros on partitions 64:128)
            kT_stack = []
            for h in range(HPC):
                kst = ph3.tile([128, 2 * S], FP8, tag=f"kst{h}", name="kst")
                nc.gpsimd.memset(kst[64:128, S:2 * S], 0.0)
                nc.scalar.copy(out=kst[0:64, S:2 * S], in_=krTr)
                kT_stack.append(kst)
            nkt = KVLR // 128
            for h in range(HPC):
                for ch in range(4):
                    kp = ps3.tile([128, 512], F32, tag="mmk", bufs=2,
                                  name="kp")
                    for kt in range(nkt):
                        nc.tensor.matmul(
                            out=kp,
                            lhsT=wkvb[:, kt * NKV + h * DN:
                                      kt * NKV + (h + 1) * DN],
                            rhs=xk[kt][:, _ts(ch, 512)],
                            start=(kt == 0), stop=(kt == nkt - 1))
                    nc.scalar.copy(out=kT_stack[h][:, _ts(ch, 512)], in_=kp)
            # v row-major, both heads packed: v_pack[kc] [128, 2*DV]
            v_pack = []
            for kc in range(NT):
                vp = ps3.tile([128, HPC * DV], F32, tag="mmv", bufs=2,
                              name="vp")
                for kt in range(nkt):
                    nc.tensor.matmul(
                        out=vp, lhsT=xk[kt][:, _ts(kc, 128)],
                        rhs=wkvb[:, kt * NKV + HPC * DN:(kt + 1) * NKV],
                        start=(kt == 0), stop=(kt == nkt - 1))
                v_t = ph3.tile([128, HPC * DV], BF16, tag=f"v{kc}",
                               name="v_t")
                nc.vector.tensor_copy(out=v_t, in_=vp)
                v_pack.append(v_t)

            # qT halves from the fp8 AllToAll: [128, 2*1024] per (half, h)
            qT = {}
            for h in range(HPC):
                for x, off in (("E", 0), ("O", 128)):
                    qt_t = ph3.tile([128, 2 * 1024], FP8, tag=f"qT{x}{h}",
                                    name="qt_t")
                    nc.gpsimd.memset(qt_t[64:128, 1024:2048], 0.0)
                    srcn = bass.AP(
                        tensor=a2aq_out.tensor, offset=h * DQK * SPC + off,
                        ap=[[SPC, 128], [HPC * DQK * SPC, NC], [1, 128]])
                    nc.sync.dma_start(
                        out=qt_t[:, 0:1024].rearrange(
                            "p (i c) -> p i c", i=NC),
                        in_=srcn)
                    srcq = bass.AP(
                        tensor=a2aq_out.tensor,
                        offset=(h * DQK + DN) * SPC + off,
                        ap=[[SPC, 64], [HPC * DQK * SPC, NC], [1, 128]])
                    nc.sync.dma_start(
                        out=qt_t[0:64, 1024:2048].rearrange(
                            "p (i c) -> p i c", i=NC),
                        in_=srcq)
                    qT[(x, h)] = qt_t

        # =================== phase 4: attention ===================
        halves = {"E": list(range(0, NT, 2)), "O": list(range(1, NT, 2))}
        warm(130, "b")  # bridge the A2A-q wait after the kv build

        def attn_half(x):
            tiles = halves[x]
            kcmax = tiles[-1]
            xi = 0 if x == "E" else 1
            with tc.tile_pool(name=f"at{x}", bufs=1) as atp, \
                 tc.tile_pool(name=f"ps4{x}", bufs=1, space="PSUM") as ps4:
                stage = {}
                lastA = max(kc for kc in range(kcmax + 1)
                            if 128 * len([t for t in tiles if t >= kc]) > 512)
                for h in range(HPC):
                    pv_ps = ps4.tile([128, 1024], F32, tag="pv", bufs=1,
                                     name="pv_ps")
                    den_ps = ps4.tile([1, 1024], F32, tag="den", bufs=1,
                                      name="den_ps")
                    kT2 = kT_stack[h].rearrange("p (two s) -> p two s", two=2)
                    qT2 = qT[(x, h)].rearrange("p (two s) -> p two s", two=2)
                    for kc in range(kcmax + 1):
                        n_t = len([t for t in tiles if t >= kc])
                        W = 128 * n_t
                        off = 1024 - W
                        pieces = ([(off, 512, lastA)] if off < 512 else []) \
                            + [(max(off, 512), 1024, kcmax)]
                        sc = ps4.tile([128, 1024], F32, tag="sc", bufs=2,
                                      name="sc")
                        for (o2, e2, _l) in pieces:
                            nc.tensor.matmul(
                                out=sc[:, o2:e2],
                                lhsT=kT2[:, :, _ts(kc, 128)],
                                rhs=qT2[:, :, o2:e2],
                                start=True, stop=True, perf_mode=DRMODE)
                        ex = atp.tile([128, 1024], BF16, tag="ex", bufs=3,
                                      name="ex")
                        nc.scalar.activation(out=ex[:, off:1024],
                                             in_=sc[:, off:1024],
                                             func=AF.Exp,
                                             scale=1.0 / Q_SCALE)
                        if kc in tiles:
                            # diagonal: zero the strictly-upper part
                            nc.gpsimd.affine_select(
                                out=ex[:, off:off + 128],
                                in_=ex[:, off:off + 128],
                                compare_op=mybir.AluOpType.is_ge,
                                fill=0.0, base=0, pattern=[[1, 128]],
                                channel_multiplier=-1)
                        for (o2, e2, lst) in pieces:
                            nc.tensor.matmul(
                                out=den_ps[:, o2:e2], lhsT=ones_t,
                                rhs=ex[:, o2:e2], start=(kc == 0),
                                stop=(kc == lst))
                            nc.tensor.matmul(
                                out=pv_ps[:, o2:e2],
                                lhsT=v_pack[kc][:, _ts(h, DV)],
                                rhs=ex[:, o2:e2], start=(kc == 0),
                                stop=(kc == lst))
                    # normalize: broadcast denom, reciprocal, multiply
                    den_s = atp.tile([1, 1024], F32, tag="dens", bufs=2,
                                     name="den_s")
                    nc.scalar.copy(out=den_s, in_=den_ps)
                    den_b = atp.tile([128, 1024], F32, tag="denb", bufs=2,
                                     name="den_b")
                    nc.gpsimd.partition_broadcast(den_b, den_s)
                    rec = atp.tile([128, 1024], F32, tag="rec", bufs=2,
                                   name="rec")
                    nc.vector.reciprocal(out=rec, in_=den_b)
                    st_t = atp.tile([128, 1024], BF16, tag=f"stg{h}",
                                    name="st_t")
                    nc.vector.tensor_mul(st_t, pv_ps, rec)
                    stage[h] = st_t
                # stage normalized attnT -> a2ao
                for h in range(HPC):
                    dsta = bass.AP(
                        tensor=a2ao_in[xi].tensor, offset=h * DV * 128,
                        ap=[[128, 128], [HPC * DV * 128, NC], [1, 128]])
                    nc.scalar.dma_start(
                        out=dsta,
                        in_=stage[h].rearrange("p (j c) -> p j c", j=NC))
                collective("AllToAll", a2ao_in[xi], a2ao_out[xi])

        def o_proj_half(x):
            xi = 0 if x == "E" else 1
            with tc.tile_pool(name=f"op{x}", bufs=1) as opp, \
                 tc.tile_pool(name=f"pso{x}", bufs=1, space="PSUM") as pso:
                at_t = opp.tile([128, NT * 128], BF16, tag="at", name="at_t")
                srca = bass.AP(tensor=a2ao_out[xi].tensor, offset=0,
                               ap=[[128, 128], [128 * 128, NT], [1, 128]])
                nc.sync.dma_start(
                    out=at_t.rearrange("p (g c) -> p g c", g=NT), in_=srca)
                o_ps = pso.tile([128, H], F32, tag="o", bufs=1, name="o_ps")
                for g in range(NT):
                    for hb in range(H // 512):
                        nc.tensor.matmul(
                            out=o_ps[:, _ts(hb, 512)],
                            lhsT=at_t[:, _ts(g, 128)],
                            rhs=wo_sb[g // 8][:, (g % 8) * H + hb * 512:
                                              (g % 8) * H + (hb + 1) * 512],
                            start=(g == 0), stop=(g == NT - 1))
                o_sb = opp.tile([128, H], F32, tag="osb", name="o_sb")
                for hb in range(H // 512):
                    if hb % 2 == 0:
                        nc.vector.tensor_copy(out=o_sb[:, _ts(hb, 512)],
                                              in_=o_ps[:, _ts(hb, 512)])
                    else:
                        nc.scalar.copy(out=o_sb[:, _ts(hb, 512)],
                                       in_=o_ps[:, _ts(hb, 512)])
                nc.scalar.dma_start(out=out_d[_ts(xi, 128), :], in_=o_sb)

        attn_half("E")
        attn_half("O")
        warm(40, "c")  # bridge the gap until the E-half AllToAll lands
        o_proj_half("E")
        warm(70, "d")  # bridge the gap until the O-half AllToAll lands
        o_proj_half("O")

    nc.compile()
    return nc


def _prep(hidden_states, cos, sin, wq_a, q_ln, wq_b, wkv_a, kv_ln, wkv_b, wo):
    """Host-side sharding + weight prep: pre-transpose hidden, fold layernorm
    weights + softmax scale (+ fp8 headroom scale) into wq_b, reorder wkv_b
    as [k_h0|k_h1|v_h0|v_h1] per core."""
    bf = ml_dtypes.bfloat16
    hs = hidden_states.reshape(S, H)
    cos2 = np.ascontiguousarray(cos.reshape(S, DR).astype(np.float32))
    sin2 = np.ascontiguousarray(sin.reshape(S, DR).astype(np.float32))
    wqa = wq_a.astype(bf)
    wkva = wkv_a.astype(bf)
    scale = np.float32(DQK) ** np.float32(-0.5)
    wqb = (wq_b * q_ln[:, None] * scale * np.float32(Q_SCALE)).astype(bf)
    wkvb = wkv_b * kv_ln[:, None]
    wob = wo.astype(bf)

    in_maps = []
    for c in range(NC):
        r = slice(c * SPC, (c + 1) * SPC)
        wkvb_c = np.empty((KVLR, HPC * (DN + DV)), np.float32)
        for h in range(HPC):
            g = c * HPC + h
            wkvb_c[:, h * DN:(h + 1) * DN] = \
                wkvb[:, g * (DN + DV):g * (DN + DV) + DN]
            wkvb_c[:, HPC * DN + h * DV:HPC * DN + (h + 1) * DV] = \
                wkvb[:, g * (DN + DV) + DN:(g + 1) * (DN + DV)]
        in_maps.append({
            "hsT": np.ascontiguousarray(hs[r].T.astype(bf)),
            "cosr": np.ascontiguousarray(cos2[r]),
            "sinr": np.ascontiguousarray(sin2[r]),
            "wqa": wqa,
            "wkva": wkva,
            "wqb": wqb,
            "wkvb": np.ascontiguousarray(wkvb_c.astype(bf)),
            "wo": wob,
        })
    return in_maps


def kernel(**inputs) -> np.ndarray:
    if "nc" not in _CACHED:
        _CACHED["nc"] = build()
    nc = _CACHED["nc"]
    in_maps = _prep(**inputs)
    res = run_bass_kernel_spmd(nc, in_maps, list(range(NC)))
    # core c's out rows [0:128] = global q-tile 2c, [128:256] = tile 2c+1,
    # which is exactly the contiguous global block [c*256, (c+1)*256).
    out = np.concatenate([res.results[c]["out"] for c in range(NC)], axis=0)
    return out.reshape(B, S, H).astype(np.float32)


if __name__ == "__main__":
    rng = np.random.RandomState(0)
    ins = {
        "hidden_states": rng.randn(B, S, H).astype(np.float32),
        "cos": rng.rand(B, S, DR).astype(np.float32),
        "sin": rng.rand(B, S, DR).astype(np.float32),
        "wq_a": (rng.randn(H, QLR) * 0.02).astype(np.float32),
        "q_ln": np.ones(QLR, np.float32),
        "wq_b": (rng.randn(QLR, NH * DQK) * 0.02).astype(np.float32),
        "wkv_a": (rng.randn(H, KVLR + DR) * 0.02).astype(np.float32),
        "kv_ln": np.ones(KVLR, np.float32),
        "wkv_b": (rng.randn(KVLR, NH * (DN + DV)) * 0.02).astype(np.float32),
        "wo": (rng.randn(NH * DV, H) * 0.02).astype(np.float32),
    }
    out = kernel(**ins)
    print("kernel out", out.shape, out.dtype, np.abs(out).mean())

